# revision 75
# baseline (speedup 1.0000x reference)
"""Trainium2 Bass kernel for nn_Attention_49366354100559.

Multi-head attention: B=2, T=2048, D=768, H=12, Dh=64.
Reference zeroes the upper triangle of scores (not -inf) before softmax,
so masked positions contribute exp(0)=1 to the softmax — the attention
matrix is dense in attn@v.

Sharding: 8 cores = 2 batches x 4 core-groups; each core computes 3 heads
of one batch and produces a partial [2048, 768] output (pre-W_o-bias);
host sums the 4 partials per batch and adds b_o.

Per-core device program (matmul operands in fp16; PE streams 1 col/cycle):
  1. x^T is pre-transposed to fp16 on the HOST and DMA'd straight into
     SBUF feature-major (no PE transposes, no f32->f16 casts on device).
  2. q0/q1/k0/k1 projections emit first; the remaining projections
     (q2k2, V, alt2 swap, vsum) are interleaved into the J=0 score
     block so the PE and the scalar engine (exp) ramp together.
     v is token-major with an appended ones column (v_aug) so attn@v
     also accumulates the softmax denominator for free.
  3. Attention pipeline over J-major blocks: (h0,J)+(h1,J) score units
     chunk-interleaved (their K=64 matmuls land in PE row tiles T0/T8
     via base partitions 0/64); h2 alternates bases per k-tile using
     alt2. exp on ACT straight out of PSUM, causal edge fixed with
     affine_select(fill=1.0); attn@v groups of the previous block run
     between score chunk-pairs to cover exp latency. Fully-masked
     k-tiles are replaced by per-quad v column-sum suffixes (vsum).
  4. Finalize per (head, q-group): fast reciprocal of the denominator
     row, partition-broadcast, scale -> attn_out^T.
  5. O-projection per token-tile PAIR: the two K=64 wo2 matmuls use
     duplicated aout2/wo2 rows at base 64 so they alternate row tiles;
     psum evacuation split across scalar and vector engines.
"""

import os
import sys

import numpy as np

if "/opt/trn_rl_repo" not in sys.path:
    sys.path.insert(0, "/opt/trn_rl_repo")

import concourse.mybir as mybir
from concourse import bacc
from concourse.tile import TileContext
from concourse.bass_utils import run_bass_kernel_spmd

F32 = mybir.dt.float32
F16 = mybir.dt.float16
F32R = mybir.dt.float32r
AF = mybir.ActivationFunctionType
ALU = mybir.AluOpType

MODE = os.environ.get("ATTN_MMDT", "fp16")  # "fp16" | "fp32r"
MMDT = F16 if MODE == "fp16" else F32R
NPDT = np.float16 if MODE == "fp16" else np.float32

N_CORES = 8
VN = 192 if MODE == "fp16" else 256
T = 2048
D = 768
HPC = 3  # heads per core
DH = 64
NK = 16  # k-token tiles of 128
NG = 4  # q groups of 512
KT = 6  # contraction tiles for D=768


def build_nc():
    nc = bacc.Bacc("TRN2", target_bir_lowering=False, debug=False,
                   num_devices=N_CORES)
    d = {}
    d["xt"] = nc.dram_tensor("xt", [D, T], MMDT, kind="ExternalInput").ap()
    d["wqk"] = nc.dram_tensor("wqk", [D, 384], MMDT, kind="ExternalInput").ap()
    d["bqk"] = nc.dram_tensor("bqk", [128, 3], F32, kind="ExternalInput").ap()
    d["wv"] = nc.dram_tensor("wv", [D, VN], MMDT, kind="ExternalInput").ap()
    d["bv"] = nc.dram_tensor("bv", [128, VN], F32, kind="ExternalInput").ap()
    d["wo01"] = nc.dram_tensor("wo01", [128, D], MMDT,
                               kind="ExternalInput").ap()
    d["wo2"] = nc.dram_tensor("wo2", [128, D], MMDT,
                              kind="ExternalInput").ap()
    d["ones"] = nc.dram_tensor("ones", [128, 512], MMDT,
                               kind="ExternalInput").ap()
    d["y"] = nc.dram_tensor("y", [T, D], F32, kind="ExternalOutput").ap()

    with TileContext(nc) as tc:
        _emit(nc, tc, d)
    nc.compile()
    return nc


def _emit(nc, tc, d):
    from contextlib import ExitStack

    with ExitStack() as ctx:
        wp = ctx.enter_context(tc.tile_pool(name="wp", bufs=1))
        main = ctx.enter_context(tc.tile_pool(name="main", bufs=1))

        # ---- weight/constant tiles (DMAs emitted in phase 1, ordered
        # for earliest projection start) ----
        wqk = [wp.tile([128, 384], MMDT, tag=f"wqk{k}", name=f"wqk{k}")
               for k in range(KT)]
        wv = [wp.tile([128, VN], MMDT, tag=f"wv{k}", name=f"wv{k}")
              for k in range(KT)]
        wo01 = wp.tile([128, D], MMDT, tag="wo01", name="wo01")
        wo2 = wp.tile([128, D], MMDT, tag="wo2", name="wo2")  # rows 64: dup
        bqk = wp.tile([128, 3], F32, tag="bqk", name="bqk")
        bv = wp.tile([128, VN], F32, tag="bv", name="bv")
        ones = wp.tile([128, 512], MMDT, tag="ones", name="ones")

        # ---- persistent SBUF ----
        qkt = [main.tile([128, T], MMDT, tag=f"qkt{g}", name=f"qkt{g}")
               for g in range(3)]  # [q0|q1], [k0|k1], [q2|k2]
        alt2 = main.tile([128, T], MMDT, tag="alt2", name="alt2")
        vaug = [main.tile([128, NK * 65], MMDT, tag=f"vaug{h}",
                          name=f"vaug{h}") for h in range(HPC)]
        aout1 = main.tile([DH, T], MMDT, tag="aout1", name="aout1")
        aout2 = main.tile([128, T], MMDT, tag="aout2", name="aout2")
        aout01 = main.tile([128, T], MMDT, tag="aout01", name="aout01")
        accs = [[main.tile([65, 512], F32, tag=f"acc{s}{g}", name=f"acc{s}{g}")
                 for g in range(NG)] for s in range(3)]
        vsum = [[main.tile([128, 65], MMDT, tag=f"vs{h}{g}",
                           name=f"vs{h}{g}") for g in range(3)]
                for h in range(HPC)]

        # ============ phase 1: load x^T (host pre-transposed, fp16) ======
        xTp = ctx.enter_context(tc.tile_pool(name="xTp", bufs=1))
        xT = [xTp.tile([128, T], MMDT, tag=f"xT{f}", name=f"xT{f}")
              for f in range(KT)]

        qeng = [nc.scalar, nc.gpsimd, nc.sync]
        for k in range(KT):
            qeng[k % 3].dma_start(wqk[k][:], d["wqk"][k * 128:(k + 1) * 128, :])
        nc.scalar.dma_start(bqk[:], d["bqk"])
        # token-sliced so the first projection group only waits on tq=0
        for tq in range(4):
            for f in range(KT):
                nc.sync.dma_start(
                    xT[f][:, tq * 512:(tq + 1) * 512],
                    d["xt"][f * 128:(f + 1) * 128, tq * 512:(tq + 1) * 512])
        for k in range(KT):
            qeng[k % 3].dma_start(wv[k][:], d["wv"][k * 128:(k + 1) * 128, :])
        nc.scalar.dma_start(bv[:], d["bv"])
        nc.gpsimd.dma_start(ones[:], d["ones"])
        nc.sync.dma_start(wo01[:], d["wo01"])
        nc.gpsimd.dma_start(wo2[:], d["wo2"])

        # ============ phase 2+3+4: projections overlapped with attention ==
        # q0/q1/k0/k1 projections are emitted first; the remaining
        # projections (q2k2, V, vsum) are fed into the J=0 score block's
        # interleave slots so the PE and the scalar engine (exp) ramp
        # together instead of serializing the phases.
        pps_ctx = ExitStack()
        pps = None  # entered inside the attention scope, after sps (LIFO)

        def qk_proj(g, n):
            ps = pps.tile([128, 512], F32, tag="qk", name=f"qk{g}_{n}")
            for k in range(KT):
                nc.tensor.matmul(
                    ps[:], wqk[k][:, g * 128:(g + 1) * 128],
                    xT[k][:, n * 512:(n + 1) * 512],
                    start=(k == 0), stop=(k == KT - 1))
            nc.vector.tensor_scalar_add(
                qkt[g][:, n * 512:(n + 1) * 512], ps[:], bqk[:, g:g + 1])

        def v_proj(tt):
            ps = pps.tile([128, VN], F32, tag="v", name=f"v{tt}")
            for k in range(KT):
                nc.tensor.matmul(
                    ps[:], xT[k][:, tt * 128:(tt + 1) * 128], wv[k][:],
                    start=(k == 0), stop=(k == KT - 1))
            for h in range(HPC):
                nc.vector.tensor_add(
                    vaug[h][:, tt * 65:tt * 65 + 64],
                    ps[:, h * 64:(h + 1) * 64],
                    bv[:, h * 64:(h + 1) * 64])

        def alt2_swap():
            # alt2 = T3 with halves swapped (partition-shifting DMAs), so
            # h2's consecutive k-tiles can use alternating row groups
            nc.sync.dma_start(alt2[0:64, :], qkt[2][64:128, :])
            nc.sync.dma_start(alt2[64:128, :], qkt[2][0:64, :])

        def vsum_calc(h, g):
            # masked-tile V sums: vsum[h][g] = sum_{ki >= 4(g+1)} vaug_ki,
            # consumed by one extra attn@v matmul against the ones tile
            va3 = vaug[h].rearrange("p (k c) -> p c k", c=65)
            v32 = main.tile([128, 65], F32, tag="v32", name=f"v32_{h}{g}",
                            bufs=2)
            nc.vector.tensor_reduce(
                v32[:], va3[:, :, 4 * (g + 1):NK],
                axis=mybir.AxisListType.X, op=ALU.add)
            nc.vector.tensor_copy(vsum[h][g][:], v32[:])

        # ============ attention pipeline + O-projection ======
        # Score matmuls are K=64 -> the PE runs them as 64x128 row tiles
        # (tile_position auto-derived from base_partition). Two adjacent
        # score MMs at bases 0 and 64 execute CONCURRENTLY in tiles T0/T8,
        # so units are processed as (h0,J)+(h1,J) pairs with their psum
        # chunks interleaved; h2 alternates bases per k-tile via alt2.
        def qk_for(h, ki):
            if h == 0:
                return qkt[0][0:64, :], qkt[1][0:64, :]
            if h == 1:
                return qkt[0][64:128, :], qkt[1][64:128, :]
            if ki % 2 == 0:
                return qkt[2][0:64, :], alt2[0:64, :]
            return alt2[64:128, :], qkt[2][64:128, :]

        acslot = [0, 1, 2]  # accs bank per head
        fill1 = nc.gpsimd.to_reg(1.0)

        with tc.tile_pool(name="ep", bufs=22) as ep, \
             tc.tile_pool(name="fin", bufs=2) as fin, \
             tc.tile_pool(name="outp", bufs=4) as outp, \
             tc.tile_pool(name="sps", bufs=2, space="PSUM") as sps:

            late = ExitStack()
            pools = {}  # ops/oprj enter after pps closes (PSUM bank budget)
            pps = pps_ctx.enter_context(
                tc.tile_pool(name="pps", bufs=2, space="PSUM"))

            for h in range(HPC):
                nc.vector.tensor_copy(
                    vaug[h].rearrange("p (k c) -> p k c", c=65)[:, :, 64],
                    ones[:, 0:1].broadcast_to([128, NK]))
            for n in range(NG):
                qk_proj(0, n)
                qk_proj(1, n)

            proj_rest = []
            for n in range(NG):
                proj_rest.append(lambda n=n: qk_proj(2, n))
            proj_rest.append(alt2_swap)
            for tt in range(NK):
                proj_rest.append(lambda tt=tt: v_proj(tt))
            for h in range(HPC):
                for g in range(3):
                    proj_rest.append(lambda h=h, g=g: vsum_calc(h, g))

            erows = {}

            def unit_score_chunks(u):
                """Per k-tile: a list of chunk closures (one per 1024-col psum
                chunk, plus a trailing affine_select fix). Emitting chunks of
                two row-tile-alternating k-tiles back-to-back pairs them on
                the PE."""
                h, J = u
                kilists = []
                for j in range(4):
                    ki = 4 * J + j
                    qT, kT = qk_for(h, ki)
                    lo = 128 * ki
                    e = ep.tile([128, T], MMDT, tag="e", name=f"e{h}_{ki}")
                    erows[(h, ki)] = e
                    chunks = []
                    for P in range(lo // 1024, 2):

                        def chunk(P=P, e=e, qT=qT, kT=kT, lo=lo, ki=ki):
                            clo = max(lo, 1024 * P)
                            ps = sps.tile([128, 1024], F32, tag="s",
                                          name=f"s{h}_{ki}_{P}")
                            for n in range(2):
                                s0 = 1024 * P + 512 * n
                                if s0 + 512 <= lo:
                                    continue
                                a0 = max(s0, lo)  # trim masked band columns
                                nc.tensor.matmul(
                                    ps[:, a0 - 1024 * P:512 * (n + 1)],
                                    kT[:, lo:lo + 128], qT[:, a0:s0 + 512])
                            nc.scalar.activation(
                                e[:, clo:1024 * (P + 1)],
                                ps[:, clo - 1024 * P:1024], AF.Exp,
                                scale=0.125)
                        chunks.append(chunk)

                    def asfix(e=e, lo=lo, J=J):
                        w = lo + 128 - 512 * J
                        nc.gpsimd.affine_select(
                            e[:, 512 * J:lo + 128], e[:, 512 * J:lo + 128],
                            pattern=[[1, w]], compare_op=ALU.is_ge,
                            fill=fill1, base=512 * J - lo,
                            channel_multiplier=-1)
                    chunks.append(asfix)
                    kilists.append(chunks)
                return kilists

            def unit_attnv_groups(u):
                h, J = u
                groups = []
                for g in range(J, NG):

                    def grp(g=g):
                        pot = pools["ops"].tile([65, 512], F32, tag="o",
                                                name=f"o{h}{J}{g}")
                        po = pot[:]
                        has_virtual = (J == g and g < 3)
                        for j in range(4):
                            ki = 4 * J + j
                            nc.tensor.matmul(
                                po, vaug[h][:, ki * 65:ki * 65 + 65],
                                erows[(h, ki)][:, 512 * g:512 * (g + 1)],
                                start=(j == 0),
                                stop=(j == 3 and not has_virtual))
                        if has_virtual:
                            # masked k-tiles: weight-1 contribution of the
                            # precomputed V suffix sums
                            nc.tensor.matmul(po, vsum[h][g][:], ones[:],
                                             start=False, stop=True)
                        ac = accs[acslot[h]]
                        if J == 0:
                            nc.vector.tensor_copy(ac[g][:], po)
                        else:
                            nc.vector.tensor_add(ac[g][:], po, ac[g][:])
                        if J == g:
                            den = fin.tile([1, 512], F32, tag="den",
                                           name=f"den{h}{g}")
                            scr = fin.tile([1, 512], F32, tag="scr",
                                           name=f"scr{h}{g}")
                            rb = fin.tile([DH, 512], F32, tag="rb",
                                          name=f"rb{h}{g}")
                            nc.vector.tensor_copy(den[:], ac[g][64:65, :])
                            nc.vector.reciprocal_approx_fast(scr[:], den[:])
                            nc.gpsimd.partition_broadcast(rb[:], scr[:])
                            if h == 0:
                                dst = aout01[0:64, 512 * g:512 * (g + 1)]
                            elif h == 1:
                                dst = aout1[:, 512 * g:512 * (g + 1)]
                            else:
                                dst = aout2[0:64, 512 * g:512 * (g + 1)]
                            nc.vector.tensor_mul(
                                dst, ac[g][0:64, :], rb[:])
                            if h == 1:
                                # stack h1 under h0 (partition-shift DMA)
                                nc.sync.dma_start(
                                    aout01[64:128, 512 * g:512 * (g + 1)],
                                    aout1[:, 512 * g:512 * (g + 1)])
                            if h == 2:
                                # duplicate h2's rows to base 64 so o-proj
                                # wo2 matmuls can alternate row tiles
                                nc.sync.dma_start(
                                    aout2[64:128, 512 * g:512 * (g + 1)],
                                    aout2[0:64, 512 * g:512 * (g + 1)])
                                oproj_group(g)
                    groups.append(grp)
                return groups

            def oproj_group(tg):
                # token tiles processed in pairs: the two K=64 wo2 matmuls
                # alternate row tiles (T0/T8) and run concurrently
                for tp in range(2):
                    te, to = 4 * tg + 2 * tp, 4 * tg + 2 * tp + 1
                    ote = outp.tile([128, D], F32, tag="ot", name=f"ot{te}")
                    oto = outp.tile([128, D], F32, tag="ot", name=f"ot{to}")
                    for (n0, w) in ((0, 512), (512, 256)):
                        pse = pools["avp"].tile([128, 512], F32, tag="op",
                                                 name=f"op{te}_{n0}")
                        pso = pools["avp"].tile([128, 512], F32, tag="op",
                                                 name=f"op{to}_{n0}")
                        nc.tensor.matmul(
                            pse[:, 0:w], aout01[:, te * 128:(te + 1) * 128],
                            wo01[:, n0:n0 + w], start=True, stop=False)
                        nc.tensor.matmul(
                            pso[:, 0:w], aout01[:, to * 128:(to + 1) * 128],
                            wo01[:, n0:n0 + w], start=True, stop=False)
                        nc.tensor.matmul(
                            pse[:, 0:w], aout2[0:64, te * 128:(te + 1) * 128],
                            wo2[0:64, n0:n0 + w], start=False, stop=True)
                        nc.tensor.matmul(
                            pso[:, 0:w], aout2[64:128, to * 128:(to + 1) * 128],
                            wo2[64:128, n0:n0 + w], start=False, stop=True)
                        nc.vector.tensor_copy(ote[:, n0:n0 + w], pse[:, 0:w])
                        nc.vector.tensor_copy(oto[:, n0:n0 + w], pso[:, 0:w])
                    nc.sync.dma_start(d["y"][te * 128:(te + 1) * 128, :],
                                      ote[:])
                    nc.sync.dma_start(d["y"][to * 128:(to + 1) * 128, :],
                                      oto[:])

            def roundrobin(lists):
                out = []
                for i in range(max(len(l) for l in lists)):
                    for l in lists:
                        if i < len(l):
                            out.append(l[i])
                return out

            blocks = [[(0, J), (1, J), (2, J)] for J in range(4)]
            pending = proj_rest
            for bi, blk in enumerate(blocks):
                if bi == 1:
                    # all projection work emitted; swap PSUM pools
                    pps_ctx.close()
                    pools["ops"] = late.enter_context(
                        tc.tile_pool(name="ops", bufs=2, space="PSUM"))
                    pools["avp"] = late.enter_context(
                        tc.tile_pool(name="avp", bufs=2, space="PSUM"))
                kilists = [kl for u in blk for kl in unit_score_chunks(u)]
                # h0/h1 pair per k-tile (row tiles T0/T8); h2 pairs its own
                # adjacent k-tiles, whose bases alternate by ki parity
                slots = ([roundrobin([kilists[i], kilists[4 + i]])
                          for i in range(4)]
                         + [roundrobin([kilists[8], kilists[9]]),
                            roundrobin([kilists[10], kilists[11]])])
                npts = sum((len(s) + 1) // 2 for s in slots)
                per = (len(pending) + npts - 1) // npts
                gi = 0
                for slot in slots:
                    for ci in range(0, len(slot), 2):
                        slot[ci]()
                        if ci + 1 < len(slot):
                            slot[ci + 1]()
                        for _ in range(per):
                            if gi < len(pending):
                                pending[gi]()
                                gi += 1
                while gi < len(pending):
                    pending[gi]()
                    gi += 1
                pending = [g for u in blk for g in unit_attnv_groups(u)]
            for grp in pending:
                grp()
            late.close()


_NC_CACHE = None


def _get_nc():
    global _NC_CACHE
    if _NC_CACHE is None:
        _NC_CACHE = build_nc()
    return _NC_CACHE


def _make_in_maps(residual_stream, W_q, b_q, W_k, b_k, W_v, b_v, W_o, b_o):
    in_maps = []
    for c in range(N_CORES):
        b = c // 4
        hs = [3 * (c % 4) + i for i in range(HPC)]
        cs = [slice(64 * h, 64 * h + 64) for h in hs]
        wqk = np.concatenate(
            [W_q[:, cs[0]], W_q[:, cs[1]], W_k[:, cs[0]], W_k[:, cs[1]],
             W_q[:, cs[2]], W_k[:, cs[2]]], axis=1).astype(NPDT)
        bqk = np.concatenate(
            [b_q[cs[0]], b_q[cs[1]], b_k[cs[0]], b_k[cs[1]],
             b_q[cs[2]], b_k[cs[2]]]).astype(np.float32)
        bqk = np.ascontiguousarray(bqk.reshape(3, 128).T)
        wv = np.zeros((D, VN), dtype=NPDT)
        wv[:, :192] = np.concatenate([W_v[:, s] for s in cs], axis=1)
        bv = np.zeros((1, VN), dtype=np.float32)
        bv[0, :192] = np.concatenate([b_v[s] for s in cs])
        bv = np.ascontiguousarray(np.broadcast_to(bv, (128, VN)))
        m = {
            "xt": residual_stream[b].T.astype(NPDT, order="C"),
            "wqk": wqk,
            "bqk": bqk,
            "wv": wv,
            "bv": bv,
            "ones": np.ones((128, 512), dtype=NPDT),
        }
        m["wo01"] = np.ascontiguousarray(
            W_o[64 * hs[0]:64 * hs[0] + 128, :]).astype(NPDT)
        wo2h = W_o[64 * hs[2]:64 * hs[2] + 64, :].astype(NPDT)
        m["wo2"] = np.ascontiguousarray(np.concatenate([wo2h, wo2h], axis=0))
        in_maps.append(m)
    return in_maps


def kernel(residual_stream, W_q, b_q, W_k, b_k, W_v, b_v, W_o, b_o,
           _trace=False):
    residual_stream = np.asarray(residual_stream, dtype=np.float32)
    args = [np.asarray(a, dtype=np.float32)
            for a in (W_q, b_q, W_k, b_k, W_v, b_v, W_o, b_o)]
    W_q, b_q, W_k, b_k, W_v, b_v, W_o, b_o = args
    nc = _get_nc()
    in_maps = _make_in_maps(residual_stream, W_q, b_q, W_k, b_k, W_v, b_v,
                            W_o, b_o)
    res = run_bass_kernel_spmd(nc, in_maps, core_ids=list(range(N_CORES)),
                               trace=_trace)
    B = residual_stream.shape[0]
    out = np.zeros((B, T, D), dtype=np.float32)
    for c in range(N_CORES):
        out[c // 4] += res.results[c]["y"]
    out += b_o[None, None, :]
    if _trace:
        kernel._last_result = res
    return out



# revision 77
# speedup vs baseline: 1.0303x; 1.0303x over previous
"""Trainium2 Bass kernel for nn_Attention_49366354100559.

Multi-head attention: B=2, T=2048, D=768, H=12, Dh=64.
Reference zeroes the upper triangle of scores (not -inf) before softmax,
so masked positions contribute exp(0)=1 to the softmax — the attention
matrix is dense in attn@v.

Sharding: 8 cores = 2 batches x 4 core-groups; each core computes 3 heads
of one batch and produces a partial [2048, 768] output (pre-W_o-bias);
host sums the 4 partials per batch and adds b_o.

Per-core device program (matmul operands in fp16; PE streams 1 col/cycle):
  1. x^T is pre-transposed to fp16 on the HOST and DMA'd straight into
     SBUF feature-major (no PE transposes, no f32->f16 casts on device).
  2. q0/q1/k0/k1 projections emit first; the remaining projections
     (q2k2, V, alt2 swap, vsum) are interleaved into the J=0 score
     block so the PE and the scalar engine (exp) ramp together.
     v is token-major with an appended ones column (v_aug) so attn@v
     also accumulates the softmax denominator for free.
  3. Attention pipeline over J-major blocks: (h0,J)+(h1,J) score units
     chunk-interleaved (their K=64 matmuls land in PE row tiles T0/T8
     via base partitions 0/64); h2 alternates bases per k-tile using
     alt2. exp on ACT straight out of PSUM, causal edge fixed with
     affine_select(fill=1.0); attn@v groups of the previous block run
     between score chunk-pairs to cover exp latency. Fully-masked
     k-tiles are replaced by per-quad v column-sum suffixes (vsum).
  4. Finalize per (head, q-group): fast reciprocal of the denominator
     row, partition-broadcast, scale -> attn_out^T.
  5. O-projection per token-tile PAIR: the two K=64 wo2 matmuls use
     duplicated aout2/wo2 rows at base 64 so they alternate row tiles;
     psum evacuation split across scalar and vector engines.
"""

import os
import sys

import numpy as np

if "/opt/trn_rl_repo" not in sys.path:
    sys.path.insert(0, "/opt/trn_rl_repo")

import concourse.mybir as mybir
from concourse import bacc
from concourse.tile import TileContext
from concourse.bass_utils import run_bass_kernel_spmd

F32 = mybir.dt.float32
F16 = mybir.dt.float16
F32R = mybir.dt.float32r
AF = mybir.ActivationFunctionType
ALU = mybir.AluOpType

MODE = os.environ.get("ATTN_MMDT", "fp16")  # "fp16" | "fp32r"
MMDT = F16 if MODE == "fp16" else F32R
NPDT = np.float16 if MODE == "fp16" else np.float32

N_CORES = 8
VN = 192 if MODE == "fp16" else 256
T = 2048
D = 768
HPC = 3  # heads per core
DH = 64
NK = 16  # k-token tiles of 128
NG = 4  # q groups of 512
KT = 6  # contraction tiles for D=768


def build_nc():
    nc = bacc.Bacc("TRN2", target_bir_lowering=False, debug=False,
                   num_devices=N_CORES)
    d = {}
    d["xt"] = nc.dram_tensor("xt", [D, T], MMDT, kind="ExternalInput").ap()
    d["wqk"] = nc.dram_tensor("wqk", [D, 384], MMDT, kind="ExternalInput").ap()
    d["bqk"] = nc.dram_tensor("bqk", [128, 3], F32, kind="ExternalInput").ap()
    d["wv"] = nc.dram_tensor("wv", [D, VN], MMDT, kind="ExternalInput").ap()
    d["bv"] = nc.dram_tensor("bv", [128, VN], F32, kind="ExternalInput").ap()
    d["wo01"] = nc.dram_tensor("wo01", [128, D], MMDT,
                               kind="ExternalInput").ap()
    d["wo2"] = nc.dram_tensor("wo2", [128, D], MMDT,
                              kind="ExternalInput").ap()
    d["ones"] = nc.dram_tensor("ones", [128, 512], MMDT,
                               kind="ExternalInput").ap()
    d["y"] = nc.dram_tensor("y", [T, D], F32, kind="ExternalOutput").ap()

    with TileContext(nc) as tc:
        _emit(nc, tc, d)
    nc.compile()
    return nc


def _emit(nc, tc, d):
    from contextlib import ExitStack

    with ExitStack() as ctx:
        wp = ctx.enter_context(tc.tile_pool(name="wp", bufs=1))
        main = ctx.enter_context(tc.tile_pool(name="main", bufs=1))

        # ---- weight/constant tiles (DMAs emitted in phase 1, ordered
        # for earliest projection start) ----
        wqk = [wp.tile([128, 384], MMDT, tag=f"wqk{k}", name=f"wqk{k}")
               for k in range(KT)]
        wv = [wp.tile([128, VN], MMDT, tag=f"wv{k}", name=f"wv{k}")
              for k in range(KT)]
        wo01 = wp.tile([128, D], MMDT, tag="wo01", name="wo01")
        wo2 = wp.tile([128, D], MMDT, tag="wo2", name="wo2")  # rows 64: dup
        bqk = wp.tile([128, 3], F32, tag="bqk", name="bqk")
        bv = wp.tile([128, VN], F32, tag="bv", name="bv")
        ones = wp.tile([128, 512], MMDT, tag="ones", name="ones")

        # ---- persistent SBUF ----
        qkt = [main.tile([128, T], MMDT, tag=f"qkt{g}", name=f"qkt{g}")
               for g in range(3)]  # [q0|q1], [k0|k1], [q2|k2]
        alt2 = main.tile([128, T], MMDT, tag="alt2", name="alt2")
        vaug = [main.tile([128, NK * 65], MMDT, tag=f"vaug{h}",
                          name=f"vaug{h}") for h in range(HPC)]
        aout1 = main.tile([DH, T], MMDT, tag="aout1", name="aout1")
        aout2 = main.tile([128, T], MMDT, tag="aout2", name="aout2")
        aout01 = main.tile([128, T], MMDT, tag="aout01", name="aout01")
        accs = [[main.tile([65, 512], F32, tag=f"acc{s}{g}", name=f"acc{s}{g}")
                 for g in range(NG)] for s in range(3)]
        vsum = [[main.tile([128, 65], MMDT, tag=f"vs{h}{g}",
                           name=f"vs{h}{g}") for g in range(3)]
                for h in range(HPC)]

        # ============ phase 1: load x^T (host pre-transposed, fp16) ======
        xTp = ctx.enter_context(tc.tile_pool(name="xTp", bufs=1))
        xT = [xTp.tile([128, T], MMDT, tag=f"xT{f}", name=f"xT{f}")
              for f in range(KT)]

        qeng = [nc.scalar, nc.gpsimd, nc.sync]
        for k in range(KT):
            qeng[k % 3].dma_start(wqk[k][:], d["wqk"][k * 128:(k + 1) * 128, :])
        nc.scalar.dma_start(bqk[:], d["bqk"])
        # token-sliced so the first projection group only waits on tq=0
        for tq in range(4):
            for f in range(KT):
                nc.sync.dma_start(
                    xT[f][:, tq * 512:(tq + 1) * 512],
                    d["xt"][f * 128:(f + 1) * 128, tq * 512:(tq + 1) * 512])
        for k in range(KT):
            qeng[k % 3].dma_start(wv[k][:], d["wv"][k * 128:(k + 1) * 128, :])
        nc.scalar.dma_start(bv[:], d["bv"])
        nc.gpsimd.dma_start(ones[:], d["ones"])
        nc.sync.dma_start(wo01[:], d["wo01"])
        nc.gpsimd.dma_start(wo2[:], d["wo2"])

        # ============ phase 2+3+4: projections overlapped with attention ==
        # q0/q1/k0/k1 projections are emitted first; the remaining
        # projections (q2k2, V, vsum) are fed into the J=0 score block's
        # interleave slots so the PE and the scalar engine (exp) ramp
        # together instead of serializing the phases.
        pps_ctx = ExitStack()
        pps = None  # entered inside the attention scope, after sps (LIFO)

        def qk_proj(g, n):
            ps = pps.tile([128, 512], F32, tag="qk", name=f"qk{g}_{n}")
            for k in range(KT):
                nc.tensor.matmul(
                    ps[:], wqk[k][:, g * 128:(g + 1) * 128],
                    xT[k][:, n * 512:(n + 1) * 512],
                    start=(k == 0), stop=(k == KT - 1))
            nc.vector.tensor_scalar_add(
                qkt[g][:, n * 512:(n + 1) * 512], ps[:], bqk[:, g:g + 1])

        def v_proj(tt):
            ps = pps.tile([128, VN], F32, tag="v", name=f"v{tt}")
            for k in range(KT):
                nc.tensor.matmul(
                    ps[:], xT[k][:, tt * 128:(tt + 1) * 128], wv[k][:],
                    start=(k == 0), stop=(k == KT - 1))
            for h in range(HPC):
                nc.vector.tensor_add(
                    vaug[h][:, tt * 65:tt * 65 + 64],
                    ps[:, h * 64:(h + 1) * 64],
                    bv[:, h * 64:(h + 1) * 64])

        def alt2_swap():
            # alt2 = T3 with halves swapped (partition-shifting DMAs), so
            # h2's consecutive k-tiles can use alternating row groups
            nc.sync.dma_start(alt2[0:64, :], qkt[2][64:128, :])
            nc.sync.dma_start(alt2[64:128, :], qkt[2][0:64, :])

        def vsum_calc(h, g):
            # masked-tile V sums: vsum[h][g] = sum_{ki >= 4(g+1)} vaug_ki,
            # consumed by one extra attn@v matmul against the ones tile
            va3 = vaug[h].rearrange("p (k c) -> p c k", c=65)
            v32 = main.tile([128, 65], F32, tag="v32", name=f"v32_{h}{g}",
                            bufs=2)
            nc.vector.tensor_reduce(
                v32[:], va3[:, :, 4 * (g + 1):NK],
                axis=mybir.AxisListType.X, op=ALU.add)
            nc.vector.tensor_copy(vsum[h][g][:], v32[:])

        # ============ attention pipeline + O-projection ======
        # Score matmuls are K=64 -> the PE runs them as 64x128 row tiles
        # (tile_position auto-derived from base_partition). Two adjacent
        # score MMs at bases 0 and 64 execute CONCURRENTLY in tiles T0/T8,
        # so units are processed as (h0,J)+(h1,J) pairs with their psum
        # chunks interleaved; h2 alternates bases per k-tile via alt2.
        def qk_for(h, ki):
            if h == 0:
                return qkt[0][0:64, :], qkt[1][0:64, :]
            if h == 1:
                return qkt[0][64:128, :], qkt[1][64:128, :]
            if ki % 2 == 0:
                return qkt[2][0:64, :], alt2[0:64, :]
            return alt2[64:128, :], qkt[2][64:128, :]

        acslot = [0, 1, 2]  # accs bank per head
        fill1 = nc.gpsimd.to_reg(1.0)

        with tc.tile_pool(name="ep", bufs=22) as ep, \
             tc.tile_pool(name="fin", bufs=2) as fin, \
             tc.tile_pool(name="outp", bufs=3) as outp, \
             tc.tile_pool(name="sps", bufs=2, space="PSUM") as sps:

            late = ExitStack()
            pools = {}  # ops/oprj enter after pps closes (PSUM bank budget)
            pps = pps_ctx.enter_context(
                tc.tile_pool(name="pps", bufs=2, space="PSUM"))

            for h in range(HPC):
                nc.vector.tensor_copy(
                    vaug[h].rearrange("p (k c) -> p k c", c=65)[:, :, 64],
                    ones[:, 0:1].broadcast_to([128, NK]))
            for n in range(NG):
                qk_proj(0, n)
                qk_proj(1, n)

            proj_rest = []
            for n in range(NG):
                proj_rest.append(lambda n=n: qk_proj(2, n))
            proj_rest.append(alt2_swap)
            for tt in range(NK):
                proj_rest.append(lambda tt=tt: v_proj(tt))
            for h in range(HPC):
                for g in range(3):
                    proj_rest.append(lambda h=h, g=g: vsum_calc(h, g))

            erows = {}

            def unit_score_chunks(u):
                """Per k-tile: a list of chunk closures (one per 1024-col psum
                chunk, plus a trailing affine_select fix). Emitting chunks of
                two row-tile-alternating k-tiles back-to-back pairs them on
                the PE."""
                h, J = u
                kilists = []
                for j in range(4):
                    ki = 4 * J + j
                    qT, kT = qk_for(h, ki)
                    lo = 128 * ki
                    e = ep.tile([128, T], MMDT, tag="e", name=f"e{h}_{ki}")
                    erows[(h, ki)] = e
                    chunks = []
                    for P in range(lo // 1024, 2):

                        def chunk(P=P, e=e, qT=qT, kT=kT, lo=lo, ki=ki):
                            clo = max(lo, 1024 * P)
                            ps = sps.tile([128, 1024], F32, tag="s",
                                          name=f"s{h}_{ki}_{P}")
                            for n in range(2):
                                s0 = 1024 * P + 512 * n
                                if s0 + 512 <= lo:
                                    continue
                                a0 = max(s0, lo)  # trim masked band columns
                                nc.tensor.matmul(
                                    ps[:, a0 - 1024 * P:512 * (n + 1)],
                                    kT[:, lo:lo + 128], qT[:, a0:s0 + 512])
                            nc.scalar.activation(
                                e[:, clo:1024 * (P + 1)],
                                ps[:, clo - 1024 * P:1024], AF.Exp,
                                scale=0.125)
                        chunks.append(chunk)

                    def asfix(e=e, lo=lo, J=J):
                        w = lo + 128 - 512 * J
                        nc.gpsimd.affine_select(
                            e[:, 512 * J:lo + 128], e[:, 512 * J:lo + 128],
                            pattern=[[1, w]], compare_op=ALU.is_ge,
                            fill=fill1, base=512 * J - lo,
                            channel_multiplier=-1)
                    chunks.append(asfix)
                    kilists.append(chunks)
                return kilists

            def unit_attnv_groups(u):
                h, J = u
                groups = []
                for g in range(J, NG):

                    def grp(g=g):
                        pot = pools["ops"].tile([65, 512], F32, tag="o",
                                                name=f"o{h}{J}{g}")
                        po = pot[:]
                        has_virtual = (J == g and g < 3)
                        for j in range(4):
                            ki = 4 * J + j
                            nc.tensor.matmul(
                                po, vaug[h][:, ki * 65:ki * 65 + 65],
                                erows[(h, ki)][:, 512 * g:512 * (g + 1)],
                                start=(j == 0),
                                stop=(j == 3 and not has_virtual))
                        if has_virtual:
                            # masked k-tiles: weight-1 contribution of the
                            # precomputed V suffix sums
                            nc.tensor.matmul(po, vsum[h][g][:], ones[:],
                                             start=False, stop=True)
                        ac = accs[acslot[h]]
                        if J == 0:
                            nc.vector.tensor_copy(ac[g][:], po)
                        else:
                            nc.vector.tensor_add(ac[g][:], po, ac[g][:])
                        if J == g:
                            den = fin.tile([1, 512], F32, tag="den",
                                           name=f"den{h}{g}")
                            scr = fin.tile([1, 512], F32, tag="scr",
                                           name=f"scr{h}{g}")
                            rb = fin.tile([DH, 512], F32, tag="rb",
                                          name=f"rb{h}{g}")
                            nc.vector.tensor_copy(den[:], ac[g][64:65, :])
                            nc.vector.reciprocal_approx_fast(scr[:], den[:])
                            nc.gpsimd.partition_broadcast(rb[:], scr[:])
                            if h == 0:
                                dst = aout01[0:64, 512 * g:512 * (g + 1)]
                            elif h == 1:
                                dst = aout1[:, 512 * g:512 * (g + 1)]
                            else:
                                dst = aout2[0:64, 512 * g:512 * (g + 1)]
                            nc.vector.tensor_mul(
                                dst, ac[g][0:64, :], rb[:])
                            if h == 1:
                                # stack h1 under h0 (partition-shift DMA)
                                nc.sync.dma_start(
                                    aout01[64:128, 512 * g:512 * (g + 1)],
                                    aout1[:, 512 * g:512 * (g + 1)])
                            if h == 2:
                                # duplicate h2's rows to base 64 so o-proj
                                # wo2 matmuls can alternate row tiles
                                nc.sync.dma_start(
                                    aout2[64:128, 512 * g:512 * (g + 1)],
                                    aout2[0:64, 512 * g:512 * (g + 1)])
                                oproj_group(g)
                    groups.append(grp)
                return groups

            def oproj_group(tg):
                # token tiles processed in pairs: the two K=64 wo2 matmuls
                # alternate row tiles (T0/T8) and run concurrently
                for tp in range(2):
                    te, to = 4 * tg + 2 * tp, 4 * tg + 2 * tp + 1
                    ote = outp.tile([128, D], F32, tag="ot", name=f"ot{te}")
                    oto = outp.tile([128, D], F32, tag="ot", name=f"ot{to}")
                    for (n0, w) in ((0, 512), (512, 256)):
                        pse = pools["avp"].tile([128, 512], F32, tag="op",
                                                 name=f"op{te}_{n0}")
                        pso = pools["avp"].tile([128, 512], F32, tag="op",
                                                 name=f"op{to}_{n0}")
                        nc.tensor.matmul(
                            pse[:, 0:w], aout01[:, te * 128:(te + 1) * 128],
                            wo01[:, n0:n0 + w], start=True, stop=False)
                        nc.tensor.matmul(
                            pso[:, 0:w], aout01[:, to * 128:(to + 1) * 128],
                            wo01[:, n0:n0 + w], start=True, stop=False)
                        nc.tensor.matmul(
                            pse[:, 0:w], aout2[0:64, te * 128:(te + 1) * 128],
                            wo2[0:64, n0:n0 + w], start=False, stop=True)
                        nc.tensor.matmul(
                            pso[:, 0:w], aout2[64:128, to * 128:(to + 1) * 128],
                            wo2[64:128, n0:n0 + w], start=False, stop=True)
                        nc.scalar.activation(ote[:, n0:n0 + w], pse[:, 0:w],
                                             AF.Copy)
                        nc.vector.tensor_copy(oto[:, n0:n0 + w], pso[:, 0:w])
                    nc.sync.dma_start(d["y"][te * 128:(te + 1) * 128, :],
                                      ote[:])
                    nc.sync.dma_start(d["y"][to * 128:(to + 1) * 128, :],
                                      oto[:])

            def roundrobin(lists):
                out = []
                for i in range(max(len(l) for l in lists)):
                    for l in lists:
                        if i < len(l):
                            out.append(l[i])
                return out

            blocks = [[(0, J), (1, J), (2, J)] for J in range(4)]
            pending = proj_rest
            for bi, blk in enumerate(blocks):
                if bi == 1:
                    # all projection work emitted; swap PSUM pools
                    pps_ctx.close()
                    pools["ops"] = late.enter_context(
                        tc.tile_pool(name="ops", bufs=2, space="PSUM"))
                    pools["avp"] = late.enter_context(
                        tc.tile_pool(name="avp", bufs=2, space="PSUM"))
                kilists = [kl for u in blk for kl in unit_score_chunks(u)]
                # h0/h1 pair per k-tile (row tiles T0/T8); h2 pairs its own
                # adjacent k-tiles, whose bases alternate by ki parity
                slots = ([roundrobin([kilists[i], kilists[4 + i]])
                          for i in range(4)]
                         + [roundrobin([kilists[8], kilists[9]]),
                            roundrobin([kilists[10], kilists[11]])])
                npts = sum((len(s) + 1) // 2 for s in slots)
                per = (len(pending) + npts - 1) // npts
                gi = 0
                for slot in slots:
                    for ci in range(0, len(slot), 2):
                        slot[ci]()
                        if ci + 1 < len(slot):
                            slot[ci + 1]()
                        for _ in range(per):
                            if gi < len(pending):
                                pending[gi]()
                                gi += 1
                while gi < len(pending):
                    pending[gi]()
                    gi += 1
                pending = [g for u in blk for g in unit_attnv_groups(u)]
            for grp in pending:
                grp()
            late.close()


_NC_CACHE = None


def _get_nc():
    global _NC_CACHE
    if _NC_CACHE is None:
        _NC_CACHE = build_nc()
    return _NC_CACHE


def _make_in_maps(residual_stream, W_q, b_q, W_k, b_k, W_v, b_v, W_o, b_o):
    in_maps = []
    for c in range(N_CORES):
        b = c // 4
        hs = [3 * (c % 4) + i for i in range(HPC)]
        cs = [slice(64 * h, 64 * h + 64) for h in hs]
        wqk = np.concatenate(
            [W_q[:, cs[0]], W_q[:, cs[1]], W_k[:, cs[0]], W_k[:, cs[1]],
             W_q[:, cs[2]], W_k[:, cs[2]]], axis=1).astype(NPDT)
        bqk = np.concatenate(
            [b_q[cs[0]], b_q[cs[1]], b_k[cs[0]], b_k[cs[1]],
             b_q[cs[2]], b_k[cs[2]]]).astype(np.float32)
        bqk = np.ascontiguousarray(bqk.reshape(3, 128).T)
        wv = np.zeros((D, VN), dtype=NPDT)
        wv[:, :192] = np.concatenate([W_v[:, s] for s in cs], axis=1)
        bv = np.zeros((1, VN), dtype=np.float32)
        bv[0, :192] = np.concatenate([b_v[s] for s in cs])
        bv = np.ascontiguousarray(np.broadcast_to(bv, (128, VN)))
        m = {
            "xt": residual_stream[b].T.astype(NPDT, order="C"),
            "wqk": wqk,
            "bqk": bqk,
            "wv": wv,
            "bv": bv,
            "ones": np.ones((128, 512), dtype=NPDT),
        }
        m["wo01"] = np.ascontiguousarray(
            W_o[64 * hs[0]:64 * hs[0] + 128, :]).astype(NPDT)
        wo2h = W_o[64 * hs[2]:64 * hs[2] + 64, :].astype(NPDT)
        m["wo2"] = np.ascontiguousarray(np.concatenate([wo2h, wo2h], axis=0))
        in_maps.append(m)
    return in_maps


def kernel(residual_stream, W_q, b_q, W_k, b_k, W_v, b_v, W_o, b_o,
           _trace=False):
    residual_stream = np.asarray(residual_stream, dtype=np.float32)
    args = [np.asarray(a, dtype=np.float32)
            for a in (W_q, b_q, W_k, b_k, W_v, b_v, W_o, b_o)]
    W_q, b_q, W_k, b_k, W_v, b_v, W_o, b_o = args
    nc = _get_nc()
    in_maps = _make_in_maps(residual_stream, W_q, b_q, W_k, b_k, W_v, b_v,
                            W_o, b_o)
    res = run_bass_kernel_spmd(nc, in_maps, core_ids=list(range(N_CORES)),
                               trace=_trace)
    B = residual_stream.shape[0]
    out = np.zeros((B, T, D), dtype=np.float32)
    for c in range(N_CORES):
        out[c // 4] += res.results[c]["y"]
    out += b_o[None, None, :]
    if _trace:
        kernel._last_result = res
    return out



# revision 78
# speedup vs baseline: 1.2257x; 1.1896x over previous
"""Trainium2 Bass kernel for nn_Attention_49366354100559.

Multi-head attention: B=2, T=2048, D=768, H=12, Dh=64.
Reference zeroes the upper triangle of scores (not -inf) before softmax,
so masked positions contribute exp(0)=1 to the softmax — the attention
matrix is dense in attn@v.

Sharding: 8 cores = 2 batches x 4 core-groups; each core computes 3 heads
of one batch and produces a partial [2048, 768] output (pre-W_o-bias);
host sums the 4 partials per batch and adds b_o.

Per-core device program (matmul operands in fp16; PE streams 1 col/cycle):
  1. x^T is pre-transposed to fp16 on the HOST and DMA'd straight into
     SBUF feature-major (no PE transposes, no f32->f16 casts on device).
  2. q0/q1/k0/k1 projections emit first; the remaining projections
     (q2k2, V, alt2 swap, vsum) are interleaved into the J=0 score
     block so the PE and the scalar engine (exp) ramp together.
     v is token-major with an appended ones column (v_aug) so attn@v
     also accumulates the softmax denominator for free.
  3. Attention pipeline over J-major blocks: (h0,J)+(h1,J) score units
     chunk-interleaved (their K=64 matmuls land in PE row tiles T0/T8
     via base partitions 0/64); h2 alternates bases per k-tile using
     alt2. exp on ACT straight out of PSUM, causal edge fixed with
     affine_select(fill=1.0); attn@v groups of the previous block run
     between score chunk-pairs to cover exp latency. Fully-masked
     k-tiles are replaced by per-quad v column-sum suffixes (vsum).
  4. Finalize per (head, q-group): fast reciprocal of the denominator
     row, partition-broadcast, scale -> attn_out^T.
  5. O-projection per token-tile PAIR: the two K=64 wo2 matmuls use
     duplicated aout2/wo2 rows at base 64 so they alternate row tiles;
     psum evacuation split across scalar and vector engines.
"""

import os
import sys

import numpy as np

if "/opt/trn_rl_repo" not in sys.path:
    sys.path.insert(0, "/opt/trn_rl_repo")

import concourse.mybir as mybir
from concourse import bacc
from concourse.tile import TileContext
from concourse.bass_utils import run_bass_kernel_spmd

F32 = mybir.dt.float32
F16 = mybir.dt.float16
F32R = mybir.dt.float32r
AF = mybir.ActivationFunctionType
ALU = mybir.AluOpType

MODE = os.environ.get("ATTN_MMDT", "fp16")  # "fp16" | "fp32r"
MMDT = F16 if MODE == "fp16" else F32R
NPDT = np.float16 if MODE == "fp16" else np.float32

N_CORES = 8
VN = 192 if MODE == "fp16" else 256
T = 2048
D = 768
HPC = 3  # heads per core
DH = 64
NK = 16  # k-token tiles of 128
NG = 4  # q groups of 512
KT = 6  # contraction tiles for D=768


def build_nc():
    nc = bacc.Bacc("TRN2", target_bir_lowering=False, debug=False,
                   num_devices=N_CORES)
    d = {}
    d["xt"] = nc.dram_tensor("xt", [D, T], MMDT, kind="ExternalInput").ap()
    d["wqk"] = nc.dram_tensor("wqk", [D, 384], MMDT, kind="ExternalInput").ap()
    d["bqk"] = nc.dram_tensor("bqk", [128, 3], F32, kind="ExternalInput").ap()
    d["wv"] = nc.dram_tensor("wv", [D, VN], MMDT, kind="ExternalInput").ap()
    d["bv"] = nc.dram_tensor("bv", [128, VN], F32, kind="ExternalInput").ap()
    d["wo01"] = nc.dram_tensor("wo01", [128, D], MMDT,
                               kind="ExternalInput").ap()
    d["wo2"] = nc.dram_tensor("wo2", [128, D], MMDT,
                              kind="ExternalInput").ap()
    d["ones"] = nc.dram_tensor("ones", [128, 512], MMDT,
                               kind="ExternalInput").ap()
    d["y"] = nc.dram_tensor("y", [T, D], F32, kind="ExternalOutput").ap()

    with TileContext(nc) as tc:
        _emit(nc, tc, d)
    nc.compile()
    return nc


def _emit(nc, tc, d):
    from contextlib import ExitStack

    with ExitStack() as ctx:
        wp = ctx.enter_context(tc.tile_pool(name="wp", bufs=1))
        main = ctx.enter_context(tc.tile_pool(name="main", bufs=1))

        # ---- weight/constant tiles (DMAs emitted in phase 1, ordered
        # for earliest projection start) ----
        wqk = [wp.tile([128, 384], MMDT, tag=f"wqk{k}", name=f"wqk{k}")
               for k in range(KT)]
        wv = [wp.tile([128, VN], MMDT, tag=f"wv{k}", name=f"wv{k}")
              for k in range(KT)]
        wo01 = wp.tile([128, D], MMDT, tag="wo01", name="wo01")
        wo2 = wp.tile([128, D], MMDT, tag="wo2", name="wo2")  # rows 64: dup
        bqk = wp.tile([128, 3], F32, tag="bqk", name="bqk")
        bv = wp.tile([128, VN], F32, tag="bv", name="bv")
        ones = wp.tile([128, 512], MMDT, tag="ones", name="ones")

        # ---- persistent SBUF ----
        qkt = [main.tile([128, T], MMDT, tag=f"qkt{g}", name=f"qkt{g}")
               for g in range(3)]  # [q0|q1], [k0|k1], [q2|k2]
        alt2 = main.tile([128, T], MMDT, tag="alt2", name="alt2")
        vaug = [main.tile([128, NK * 65], MMDT, tag=f"vaug{h}",
                          name=f"vaug{h}") for h in range(HPC)]
        aout1 = main.tile([DH, T], MMDT, tag="aout1", name="aout1")
        aout2 = main.tile([128, T], MMDT, tag="aout2", name="aout2")
        aout01 = main.tile([128, T], MMDT, tag="aout01", name="aout01")
        accs = [[main.tile([65, 512], F32, tag=f"acc{s}{g}", name=f"acc{s}{g}")
                 for g in range(NG)] for s in range(3)]
        vsum = [[main.tile([128, 65], MMDT, tag=f"vs{h}{g}",
                           name=f"vs{h}{g}") for g in range(3)]
                for h in range(HPC)]

        # ============ phase 1: load x^T (host pre-transposed, fp16) ======
        xTp = ctx.enter_context(tc.tile_pool(name="xTp", bufs=1))
        xT = [xTp.tile([128, T], MMDT, tag=f"xT{f}", name=f"xT{f}")
              for f in range(KT)]

        # critical path for the first QK matmul: xT tq0 (sync queue) and
        # wqk (scalar/gpsimd, so it doesn't queue behind xT) land first
        for f in range(KT):
            nc.sync.dma_start(xT[f][:, 0:512], d["xt"][f * 128:(f + 1) * 128,
                                                       0:512])
        qeng = [nc.scalar, nc.gpsimd]
        for k in range(KT):
            qeng[k % 2].dma_start(wqk[k][:], d["wqk"][k * 128:(k + 1) * 128, :])
        nc.scalar.dma_start(bqk[:], d["bqk"])
        for tq in range(1, 4):
            for f in range(KT):
                nc.sync.dma_start(
                    xT[f][:, tq * 512:(tq + 1) * 512],
                    d["xt"][f * 128:(f + 1) * 128, tq * 512:(tq + 1) * 512])
        for k in range(KT):
            qeng[k % 2].dma_start(wv[k][:], d["wv"][k * 128:(k + 1) * 128, :])
        nc.scalar.dma_start(bv[:], d["bv"])
        nc.gpsimd.dma_start(ones[:], d["ones"])
        nc.sync.dma_start(wo01[:], d["wo01"])
        nc.gpsimd.dma_start(wo2[:], d["wo2"])

        # ============ phase 2+3+4: projections overlapped with attention ==
        # q0/q1/k0/k1 projections are emitted first; the remaining
        # projections (q2k2, V, vsum) are fed into the J=0 score block's
        # interleave slots so the PE and the scalar engine (exp) ramp
        # together instead of serializing the phases.
        pps_ctx = ExitStack()
        pps = None  # entered inside the attention scope, after sps (LIFO)

        def qk_proj(g, n):
            ps = pps.tile([128, 512], F32, tag="qk", name=f"qk{g}_{n}")
            for k in range(KT):
                nc.tensor.matmul(
                    ps[:], wqk[k][:, g * 128:(g + 1) * 128],
                    xT[k][:, n * 512:(n + 1) * 512],
                    start=(k == 0), stop=(k == KT - 1))
            nc.vector.tensor_scalar_add(
                qkt[g][:, n * 512:(n + 1) * 512], ps[:], bqk[:, g:g + 1])

        def v_proj(tt):
            ps = pps.tile([128, VN], F32, tag="v", name=f"v{tt}")
            for k in range(KT):
                nc.tensor.matmul(
                    ps[:], xT[k][:, tt * 128:(tt + 1) * 128], wv[k][:],
                    start=(k == 0), stop=(k == KT - 1))
            for h in range(HPC):
                nc.vector.tensor_add(
                    vaug[h][:, tt * 65:tt * 65 + 64],
                    ps[:, h * 64:(h + 1) * 64],
                    bv[:, h * 64:(h + 1) * 64])

        def alt2_swap():
            # alt2 = T3 with halves swapped (partition-shifting DMAs), so
            # h2's consecutive k-tiles can use alternating row groups
            nc.sync.dma_start(alt2[0:64, :], qkt[2][64:128, :])
            nc.sync.dma_start(alt2[64:128, :], qkt[2][0:64, :])

        def vsum_calc(h, g):
            # masked-tile V sums: vsum[h][g] = sum_{ki >= 4(g+1)} vaug_ki,
            # consumed by one extra attn@v matmul against the ones tile
            va3 = vaug[h].rearrange("p (k c) -> p c k", c=65)
            v32 = main.tile([128, 65], F32, tag="v32", name=f"v32_{h}{g}",
                            bufs=2)
            nc.vector.tensor_reduce(
                v32[:], va3[:, :, 4 * (g + 1):NK],
                axis=mybir.AxisListType.X, op=ALU.add)
            nc.vector.tensor_copy(vsum[h][g][:], v32[:])

        # ============ attention pipeline + O-projection ======
        # Score matmuls are K=64 -> the PE runs them as 64x128 row tiles
        # (tile_position auto-derived from base_partition). Two adjacent
        # score MMs at bases 0 and 64 execute CONCURRENTLY in tiles T0/T8,
        # so units are processed as (h0,J)+(h1,J) pairs with their psum
        # chunks interleaved; h2 alternates bases per k-tile via alt2.
        def qk_for(h, ki):
            if h == 0:
                return qkt[0][0:64, :], qkt[1][0:64, :]
            if h == 1:
                return qkt[0][64:128, :], qkt[1][64:128, :]
            if ki % 2 == 0:
                return qkt[2][0:64, :], alt2[0:64, :]
            return alt2[64:128, :], qkt[2][64:128, :]

        acslot = [0, 1, 2]  # accs bank per head
        fill1 = nc.gpsimd.to_reg(1.0)

        with tc.tile_pool(name="ep", bufs=22) as ep, \
             tc.tile_pool(name="fin", bufs=2) as fin, \
             tc.tile_pool(name="outp", bufs=3) as outp, \
             tc.tile_pool(name="sps", bufs=2, space="PSUM") as sps:

            late = ExitStack()
            pools = {}  # ops/oprj enter after pps closes (PSUM bank budget)
            pps = pps_ctx.enter_context(
                tc.tile_pool(name="pps", bufs=2, space="PSUM"))

            for h in range(HPC):
                nc.vector.tensor_copy(
                    vaug[h].rearrange("p (k c) -> p k c", c=65)[:, :, 64],
                    ones[:, 0:1].broadcast_to([128, NK]))
            for n in range(NG):
                qk_proj(0, n)
                qk_proj(1, n)

            proj_rest = []
            for n in range(NG):
                proj_rest.append(lambda n=n: qk_proj(2, n))
            proj_rest.append(alt2_swap)
            for tt in range(NK):
                proj_rest.append(lambda tt=tt: v_proj(tt))
            for h in range(HPC):
                for g in range(3):
                    proj_rest.append(lambda h=h, g=g: vsum_calc(h, g))

            erows = {}

            def unit_score_chunks(u):
                """Per k-tile: a list of chunk closures (one per 1024-col psum
                chunk, plus a trailing affine_select fix). Emitting chunks of
                two row-tile-alternating k-tiles back-to-back pairs them on
                the PE."""
                h, J = u
                kilists = []
                for j in range(4):
                    ki = 4 * J + j
                    qT, kT = qk_for(h, ki)
                    lo = 128 * ki
                    e = ep.tile([128, T], MMDT, tag="e", name=f"e{h}_{ki}")
                    erows[(h, ki)] = e
                    chunks = []
                    for P in range(lo // 1024, 2):

                        def chunk(P=P, e=e, qT=qT, kT=kT, lo=lo, ki=ki):
                            clo = max(lo, 1024 * P)
                            ps = sps.tile([128, 1024], F32, tag="s",
                                          name=f"s{h}_{ki}_{P}")
                            for n in range(2):
                                s0 = 1024 * P + 512 * n
                                if s0 + 512 <= lo:
                                    continue
                                a0 = max(s0, lo)  # trim masked band columns
                                nc.tensor.matmul(
                                    ps[:, a0 - 1024 * P:512 * (n + 1)],
                                    kT[:, lo:lo + 128], qT[:, a0:s0 + 512])
                            nc.scalar.activation(
                                e[:, clo:1024 * (P + 1)],
                                ps[:, clo - 1024 * P:1024], AF.Exp,
                                scale=0.125)
                        chunks.append(chunk)

                    def asfix(e=e, lo=lo, J=J):
                        w = lo + 128 - 512 * J
                        nc.gpsimd.affine_select(
                            e[:, 512 * J:lo + 128], e[:, 512 * J:lo + 128],
                            pattern=[[1, w]], compare_op=ALU.is_ge,
                            fill=fill1, base=512 * J - lo,
                            channel_multiplier=-1)
                    chunks.append(asfix)
                    kilists.append(chunks)
                return kilists

            def unit_attnv_groups(u):
                h, J = u
                groups = []
                for g in range(J, NG):

                    def grp(g=g):
                        pot = pools["ops"].tile([65, 512], F32, tag="o",
                                                name=f"o{h}{J}{g}")
                        po = pot[:]
                        has_virtual = (J == g and g < 3)
                        for j in range(4):
                            ki = 4 * J + j
                            nc.tensor.matmul(
                                po, vaug[h][:, ki * 65:ki * 65 + 65],
                                erows[(h, ki)][:, 512 * g:512 * (g + 1)],
                                start=(j == 0),
                                stop=(j == 3 and not has_virtual))
                        if has_virtual:
                            # masked k-tiles: weight-1 contribution of the
                            # precomputed V suffix sums
                            nc.tensor.matmul(po, vsum[h][g][:], ones[:],
                                             start=False, stop=True)
                        ac = accs[acslot[h]]
                        if J == 0:
                            nc.vector.tensor_copy(ac[g][:], po)
                        else:
                            nc.vector.tensor_add(ac[g][:], po, ac[g][:])
                        if J == g:
                            den = fin.tile([1, 512], F32, tag="den",
                                           name=f"den{h}{g}")
                            scr = fin.tile([1, 512], F32, tag="scr",
                                           name=f"scr{h}{g}")
                            rb = fin.tile([DH, 512], F32, tag="rb",
                                          name=f"rb{h}{g}")
                            nc.vector.tensor_copy(den[:], ac[g][64:65, :])
                            nc.vector.reciprocal_approx_fast(scr[:], den[:])
                            nc.gpsimd.partition_broadcast(rb[:], scr[:])
                            if h == 0:
                                dst = aout01[0:64, 512 * g:512 * (g + 1)]
                            elif h == 1:
                                dst = aout1[:, 512 * g:512 * (g + 1)]
                            else:
                                dst = aout2[0:64, 512 * g:512 * (g + 1)]
                            nc.vector.tensor_mul(
                                dst, ac[g][0:64, :], rb[:])
                            if h == 1:
                                # stack h1 under h0 (partition-shift DMA)
                                nc.sync.dma_start(
                                    aout01[64:128, 512 * g:512 * (g + 1)],
                                    aout1[:, 512 * g:512 * (g + 1)])
                            if h == 2:
                                # duplicate h2's rows to base 64 so o-proj
                                # wo2 matmuls can alternate row tiles
                                nc.sync.dma_start(
                                    aout2[64:128, 512 * g:512 * (g + 1)],
                                    aout2[0:64, 512 * g:512 * (g + 1)])
                                oproj_group(g)
                    groups.append(grp)
                return groups

            def oproj_group(tg):
                # token tiles processed in pairs: the two K=64 wo2 matmuls
                # alternate row tiles (T0/T8) and run concurrently
                for tp in range(2):
                    te, to = 4 * tg + 2 * tp, 4 * tg + 2 * tp + 1
                    ote = outp.tile([128, D], F32, tag="ot", name=f"ot{te}")
                    oto = outp.tile([128, D], F32, tag="ot", name=f"ot{to}")
                    for (n0, w) in ((0, 512), (512, 256)):
                        pse = pools["avp"].tile([128, 512], F32, tag="op",
                                                 name=f"op{te}_{n0}")
                        pso = pools["avp"].tile([128, 512], F32, tag="op",
                                                 name=f"op{to}_{n0}")
                        nc.tensor.matmul(
                            pse[:, 0:w], aout01[:, te * 128:(te + 1) * 128],
                            wo01[:, n0:n0 + w], start=True, stop=False)
                        nc.tensor.matmul(
                            pso[:, 0:w], aout01[:, to * 128:(to + 1) * 128],
                            wo01[:, n0:n0 + w], start=True, stop=False)
                        nc.tensor.matmul(
                            pse[:, 0:w], aout2[0:64, te * 128:(te + 1) * 128],
                            wo2[0:64, n0:n0 + w], start=False, stop=True)
                        nc.tensor.matmul(
                            pso[:, 0:w], aout2[64:128, to * 128:(to + 1) * 128],
                            wo2[64:128, n0:n0 + w], start=False, stop=True)
                        nc.scalar.activation(ote[:, n0:n0 + w], pse[:, 0:w],
                                             AF.Copy)
                        nc.vector.tensor_copy(oto[:, n0:n0 + w], pso[:, 0:w])
                    nc.sync.dma_start(d["y"][te * 128:(te + 1) * 128, :],
                                      ote[:])
                    nc.sync.dma_start(d["y"][to * 128:(to + 1) * 128, :],
                                      oto[:])

            def roundrobin(lists):
                out = []
                for i in range(max(len(l) for l in lists)):
                    for l in lists:
                        if i < len(l):
                            out.append(l[i])
                return out

            blocks = [[(0, J), (1, J), (2, J)] for J in range(4)]
            pending = proj_rest
            for bi, blk in enumerate(blocks):
                if bi == 1:
                    # all projection work emitted; swap PSUM pools
                    pps_ctx.close()
                    pools["ops"] = late.enter_context(
                        tc.tile_pool(name="ops", bufs=2, space="PSUM"))
                    pools["avp"] = late.enter_context(
                        tc.tile_pool(name="avp", bufs=2, space="PSUM"))
                kilists = [kl for u in blk for kl in unit_score_chunks(u)]
                # h0/h1 pair per k-tile (row tiles T0/T8); h2 pairs its own
                # adjacent k-tiles, whose bases alternate by ki parity
                slots = ([roundrobin([kilists[i], kilists[4 + i]])
                          for i in range(4)]
                         + [roundrobin([kilists[8], kilists[9]]),
                            roundrobin([kilists[10], kilists[11]])])
                npts = sum((len(s) + 1) // 2 for s in slots)
                per = (len(pending) + npts - 1) // npts
                gi = 0
                for slot in slots:
                    for ci in range(0, len(slot), 2):
                        slot[ci]()
                        if ci + 1 < len(slot):
                            slot[ci + 1]()
                        for _ in range(per):
                            if gi < len(pending):
                                pending[gi]()
                                gi += 1
                while gi < len(pending):
                    pending[gi]()
                    gi += 1
                pending = [g for u in blk for g in unit_attnv_groups(u)]
            for grp in pending:
                grp()
            late.close()


_NC_CACHE = None


def _get_nc():
    global _NC_CACHE
    if _NC_CACHE is None:
        _NC_CACHE = build_nc()
    return _NC_CACHE


def _make_in_maps(residual_stream, W_q, b_q, W_k, b_k, W_v, b_v, W_o, b_o):
    in_maps = []
    for c in range(N_CORES):
        b = c // 4
        hs = [3 * (c % 4) + i for i in range(HPC)]
        cs = [slice(64 * h, 64 * h + 64) for h in hs]
        wqk = np.concatenate(
            [W_q[:, cs[0]], W_q[:, cs[1]], W_k[:, cs[0]], W_k[:, cs[1]],
             W_q[:, cs[2]], W_k[:, cs[2]]], axis=1).astype(NPDT)
        bqk = np.concatenate(
            [b_q[cs[0]], b_q[cs[1]], b_k[cs[0]], b_k[cs[1]],
             b_q[cs[2]], b_k[cs[2]]]).astype(np.float32)
        bqk = np.ascontiguousarray(bqk.reshape(3, 128).T)
        wv = np.zeros((D, VN), dtype=NPDT)
        wv[:, :192] = np.concatenate([W_v[:, s] for s in cs], axis=1)
        bv = np.zeros((1, VN), dtype=np.float32)
        bv[0, :192] = np.concatenate([b_v[s] for s in cs])
        bv = np.ascontiguousarray(np.broadcast_to(bv, (128, VN)))
        m = {
            "xt": residual_stream[b].T.astype(NPDT, order="C"),
            "wqk": wqk,
            "bqk": bqk,
            "wv": wv,
            "bv": bv,
            "ones": np.ones((128, 512), dtype=NPDT),
        }
        m["wo01"] = np.ascontiguousarray(
            W_o[64 * hs[0]:64 * hs[0] + 128, :]).astype(NPDT)
        wo2h = W_o[64 * hs[2]:64 * hs[2] + 64, :].astype(NPDT)
        m["wo2"] = np.ascontiguousarray(np.concatenate([wo2h, wo2h], axis=0))
        in_maps.append(m)
    return in_maps


def kernel(residual_stream, W_q, b_q, W_k, b_k, W_v, b_v, W_o, b_o,
           _trace=False):
    residual_stream = np.asarray(residual_stream, dtype=np.float32)
    args = [np.asarray(a, dtype=np.float32)
            for a in (W_q, b_q, W_k, b_k, W_v, b_v, W_o, b_o)]
    W_q, b_q, W_k, b_k, W_v, b_v, W_o, b_o = args
    nc = _get_nc()
    in_maps = _make_in_maps(residual_stream, W_q, b_q, W_k, b_k, W_v, b_v,
                            W_o, b_o)
    res = run_bass_kernel_spmd(nc, in_maps, core_ids=list(range(N_CORES)),
                               trace=_trace)
    B = residual_stream.shape[0]
    out = np.zeros((B, T, D), dtype=np.float32)
    for c in range(N_CORES):
        out[c // 4] += res.results[c]["y"]
    out += b_o[None, None, :]
    if _trace:
        kernel._last_result = res
    return out

